# revision 7
# baseline (speedup 1.0000x reference)
"""Trainium2 Bass kernel for nn_Attention_4088808866132 (topk_masking).

Data-parallel over batch B=16 across 8 NeuronCores (2 batches/core).

Algebraic restructuring vs the reference:
  - Pass-1 MHA output is discarded; only head-averaged attention weights are
    needed. With a single query token the K-projection folds into the query:
        scores[b,n,h] = v_seq[b,n,:] . rq[b,h,:],   rq = (Wk_h^T qh_h)/sqrt(hd)
  - top_k(softmax(x)) == top_k(x)  (softmax is monotone), and attention is
    permutation-invariant over keys, so only the top-K *set* matters. The
    pass-2 scores are the same pass-1 scores restricted to the selected set,
    so pass 2 is a masked softmax over pass-1 scores:
        t[b,h,:]  = sum_n mask_n exp(s[b,n,h]) v[b,n,:]
        z2[b,h]   = sum_n mask_n exp(s[b,n,h])
    and the tiny output projections run on host.
Device does all O(N*E) work; host does O(E^2) postprocessing (~30 MFLOP).
"""

import numpy as np
import ml_dtypes

B, N, E, H = 16, 4096, 1024, 16
HD = E // H
K = 2048
NCORES = 8
BPC = B // NCORES          # batches per core
NT = N // 128              # 32 n-tiles per batch
EC = E // 128              # 8 e-chunks
NQ = N // 512              # 8 n-chunks for pass A
F32MAX = 0.35              # |attn_w + 0.05*noise| bound (noise would need >7 sigma)
ROUNDS = 6                 # 9-ary search rounds: 0.7 / 9^6 = 1.3e-6 resolution


def build_bass():
    import concourse.mybir as mybir
    from concourse import bacc
    from concourse.tile import TileContext
    from concourse.masks import make_identity

    dt = mybir.dt
    AF = mybir.ActivationFunctionType
    OP = mybir.AluOpType
    AX = mybir.AxisListType

    nc = bacc.Bacc()

    v_ext = nc.dram_tensor("v", (BPC, N, E), dt.float32, kind="ExternalInput")
    rqt_ext = nc.dram_tensor("rqt", (BPC, E, H), dt.float32, kind="ExternalInput")
    nst_ext = nc.dram_tensor("nst", (BPC, 128, NT), dt.float32, kind="ExternalInput")
    out_t = nc.dram_tensor("out_t", (BPC, H, E), dt.float32, kind="ExternalOutput")
    out_z = nc.dram_tensor("out_z", (BPC, H, 1), dt.float32, kind="ExternalOutput")

    with TileContext(nc) as tc:
        with (
            tc.tile_pool(name="const", bufs=1) as cpool,
            tc.tile_pool(name="vbuf", bufs=2) as vpool,
            tc.tile_pool(name="vt", bufs=2) as vtpool,
            tc.tile_pool(name="sc", bufs=2) as scpool,
            tc.tile_pool(name="small", bufs=2) as smpool,
            tc.tile_pool(name="w2p", bufs=2) as w2pool,
            tc.tile_pool(name="outp", bufs=2) as opool,
            tc.tile_pool(name="pst", bufs=2, space="PSUM") as pst,      # v-transpose banks (2)
            tc.tile_pool(name="psA", bufs=1, space="PSUM") as psA,      # scoresT acc (1)
            tc.tile_pool(name="psX", bufs=1, space="PSUM") as psX,      # score-transpose (1)
            tc.tile_pool(name="psS", bufs=1, space="PSUM") as psS,      # small shared (1)
            tc.tile_pool(name="psB", bufs=1, space="PSUM") as psB,      # pass-B acc (2+1)
        ):
            # ---- constants ----
            id_bf = cpool.tile([128, 128], dt.bfloat16)
            make_identity(nc, id_bf)
            id_f32 = cpool.tile([16, 16], dt.float32)
            make_identity(nc, id_f32)
            ones_128x1_f32 = cpool.tile([128, 1], dt.float32)
            nc.vector.memset(ones_128x1_f32, 1.0)
            ones_128x1_bf = cpool.tile([128, 1], dt.bfloat16)
            nc.vector.memset(ones_128x1_bf, 1.0)
            ones_1x128 = cpool.tile([1, 128], dt.float32)
            nc.vector.memset(ones_1x128, 1.0)
            kvec = cpool.tile([1, 8], dt.float32)
            for k in range(8):
                nc.vector.memset(kvec[:, k:k + 1], float(k + 1))

            for b in range(BPC):
                # ---- load (SWDGE casts f32->bf16 during DMA) ----
                v_bf = vpool.tile([128, NT * E], dt.bfloat16, tag="v_bf")
                for i in range(8):           # 8 DMAs x 2 MiB f32 each
                    nc.gpsimd.dma_start(
                        out=v_bf[:, i * 4 * E:(i + 1) * 4 * E]
                            .rearrange("p (t e) -> p t e", t=4),
                        in_=v_ext[b, i * 512:(i + 1) * 512, :]
                            .rearrange("(t p) e -> p t e", p=128),
                    )
                rqt_bf = smpool.tile([128, EC * H], dt.bfloat16, tag="rqt")
                nc.gpsimd.dma_start(
                    out=rqt_bf.rearrange("p (c h) -> p c h", c=EC),
                    in_=rqt_ext[b].rearrange("(c p) h -> p c h", p=128),
                )
                noise_sb = smpool.tile([128, NT], dt.float32, tag="noise")
                nc.sync.dma_start(out=noise_sb, in_=nst_ext[b])

                # ---- transpose v + pass A (scoresT) ----
                sT = scpool.tile([16, N], dt.float32, tag="sT")
                z1p = psS.tile([1, H], dt.float32, tag="sm")
                for q in range(NQ):          # 512-token chunks
                    vt = vtpool.tile([128, EC * 512], dt.bfloat16, tag="vt")
                    for c in range(EC):
                        ps = pst.tile([128, 4 * 128], dt.bfloat16, tag="tp")
                        for j in range(4):   # n-tile within chunk
                            nc.tensor.transpose(
                                ps[:, j * 128:(j + 1) * 128],
                                v_bf[:, (4 * q + j) * E + c * 128:
                                        (4 * q + j) * E + (c + 1) * 128],
                                id_bf,
                            )
                        nc.vector.tensor_copy(
                            out=vt[:, c * 512:(c + 1) * 512], in_=ps)
                    sA = psA.tile([16, 512], dt.float32, tag="sA")
                    for c in range(EC):
                        nc.tensor.matmul(
                            sA, rqt_bf[:, c * H:(c + 1) * H],
                            vt[:, c * 512:(c + 1) * 512],
                            start=(c == 0), stop=(c == EC - 1),
                        )
                    nc.scalar.copy(out=sT[:, q * 512:(q + 1) * 512], in_=sA)

                # ---- transpose scores to (n,h), exp, Z1 ----
                E2 = scpool.tile([128, NT * H], dt.float32, tag="E2")
                for g in range(4):           # groups of 8 n-tiles
                    px = psX.tile([128, 128], dt.float32, tag="px")
                    for j in range(8):
                        nc.tensor.transpose(
                            px[:, j * 16:(j + 1) * 16],
                            sT[:, (8 * g + j) * 128:(8 * g + j + 1) * 128],
                            id_f32,
                        )
                    nc.scalar.activation(
                        out=E2[:, g * 128:(g + 1) * 128], in_=px, func=AF.Exp)
                for j in range(NT):
                    nc.tensor.matmul(
                        z1p, ones_128x1_f32, E2[:, j * H:(j + 1) * H],
                        start=(j == 0), stop=(j == NT - 1),
                    )

                # ---- attn_w, noisy ----
                w16 = smpool.tile([1, H], dt.float32, tag="w16")
                nc.vector.tensor_scalar(
                    out=w16, in0=z1p, scalar1=float(H), scalar2=None, op0=OP.mult)
                nc.vector.reciprocal(w16, w16)
                wrep = psS.tile([128, H], dt.float32, tag="sm")
                nc.tensor.matmul(wrep, ones_1x128, w16, start=True, stop=True)
                awt = scpool.tile([128, NT * H], dt.float32, tag="awt")
                nc.vector.tensor_tensor(
                    out=awt.rearrange("p (t h) -> p t h", t=NT),
                    in0=E2.rearrange("p (t h) -> p t h", t=NT),
                    in1=wrep.unsqueeze(1).to_broadcast([128, NT, H]),
                    op=OP.mult)
                noisy = smpool.tile([128, NT], dt.float32, tag="noisy")
                nc.vector.tensor_reduce(
                    out=noisy, in_=awt.rearrange("p (t h) -> p t h", t=NT),
                    axis=AX.X, op=OP.add)
                nc.vector.tensor_tensor(
                    out=noisy, in0=noisy, in1=noise_sb, op=OP.add)

                # ---- 9-ary threshold search (exact top-K boundary) ----
                lo = smpool.tile([1, 1], dt.float32, tag="lo")
                stp = smpool.tile([1, 1], dt.float32, tag="stp")
                taus = smpool.tile([1, 8], dt.float32, tag="taus")
                geK = smpool.tile([1, 8], dt.float32, tag="geK")
                mm = smpool.tile([1, 1], dt.float32, tag="mm")
                ge = scpool.tile([128, 8 * NT], dt.float32, tag="ge")
                cnt = smpool.tile([128, 8], dt.float32, tag="cnt")
                nc.vector.memset(lo, -F32MAX)
                nc.vector.memset(stp, 2.0 * F32MAX / 9.0)
                for r in range(ROUNDS):
                    nc.vector.tensor_scalar(
                        out=taus, in0=kvec, scalar1=stp, scalar2=None, op0=OP.mult)
                    nc.vector.tensor_scalar(
                        out=taus, in0=taus, scalar1=lo, scalar2=None, op0=OP.add)
                    tps = psS.tile([128, 8], dt.float32, tag="sm")
                    nc.tensor.matmul(tps, ones_1x128, taus, start=True, stop=True)
                    nc.vector.tensor_tensor(
                        out=ge.rearrange("p (k t) -> p k t", k=8),
                        in0=noisy.unsqueeze(1).to_broadcast([128, 8, NT]),
                        in1=tps.unsqueeze(2).to_broadcast([128, 8, NT]),
                        op=OP.is_ge)
                    nc.vector.tensor_reduce(
                        out=cnt, in_=ge.rearrange("p (k t) -> p k t", k=8),
                        axis=AX.X, op=OP.add)
                    cps = psS.tile([1, 8], dt.float32, tag="sm")
                    nc.tensor.matmul(cps, ones_128x1_f32, cnt, start=True, stop=True)
                    nc.vector.tensor_scalar(
                        out=geK, in0=cps, scalar1=float(K), scalar2=None, op0=OP.is_ge)
                    nc.vector.tensor_reduce(
                        out=mm, in_=geK.rearrange("p (o k) -> p o k", o=1),
                        axis=AX.X, op=OP.add)
                    nc.vector.tensor_scalar(
                        out=mm, in0=mm, scalar1=stp, scalar2=None, op0=OP.mult)
                    nc.vector.tensor_tensor(out=lo, in0=lo, in1=mm, op=OP.add)
                    if r != ROUNDS - 1:
                        nc.vector.tensor_scalar(
                            out=stp, in0=stp, scalar1=1.0 / 9.0, scalar2=None,
                            op0=OP.mult)

                # ---- mask + w2 ----
                lops = psS.tile([128, 1], dt.float32, tag="sm")
                nc.tensor.matmul(lops, ones_1x128, lo, start=True, stop=True)
                maskb = smpool.tile([128, NT], dt.float32, tag="maskb")
                nc.vector.tensor_scalar(
                    out=maskb, in0=noisy, scalar1=lops, scalar2=None, op0=OP.is_ge)
                w2 = w2pool.tile([128, NT * H], dt.bfloat16, tag="w2")
                nc.vector.tensor_tensor(
                    out=w2.rearrange("p (t h) -> p t h", t=NT),
                    in0=E2.rearrange("p (t h) -> p t h", t=NT),
                    in1=maskb.unsqueeze(2).to_broadcast([128, NT, H]),
                    op=OP.mult)

                # ---- pass B ----
                tacc = psB.tile([16, E], dt.float32, tag="tacc")
                z2p = psB.tile([16, 1], dt.float32, tag="z2")
                for j in range(NT):
                    w2j = w2[:, j * H:(j + 1) * H]
                    for half in range(2):
                        nc.tensor.matmul(
                            tacc[:, half * 512:(half + 1) * 512],
                            w2j,
                            v_bf[:, j * E + half * 512:j * E + (half + 1) * 512],
                            start=(j == 0), stop=(j == NT - 1),
                        )
                    nc.tensor.matmul(
                        z2p, w2j, ones_128x1_bf,
                        start=(j == 0), stop=(j == NT - 1),
                    )

                # ---- outputs ----
                t_sb = opool.tile([16, E], dt.float32, tag="t_sb")
                nc.scalar.copy(out=t_sb, in_=tacc)
                z_sb = opool.tile([16, 1], dt.float32, tag="z_sb")
                nc.scalar.copy(out=z_sb, in_=z2p)
                nc.sync.dma_start(out=out_t[b], in_=t_sb)
                nc.sync.dma_start(out=out_z[b], in_=z_sb)

    nc.finalize()
    return nc


_NC_CACHE = None
LAST_EXEC_NS = None


def kernel(v_seq, v_global, q_seq, q_global, noise,
           in_proj_w, in_proj_b, out_proj_w, out_proj_b):
    global _NC_CACHE
    from concourse.bass_utils import run_bass_kernel_spmd

    v_seq = np.asarray(v_seq, np.float32)
    q_global = np.asarray(q_global, np.float32)
    noise = np.asarray(noise, np.float32)
    in_proj_w = np.asarray(in_proj_w, np.float32)
    in_proj_b = np.asarray(in_proj_b, np.float32)
    out_proj_w = np.asarray(out_proj_w, np.float32)
    out_proj_b = np.asarray(out_proj_b, np.float32)

    Wq, Wk, Wv = in_proj_w[:E], in_proj_w[E:2 * E], in_proj_w[2 * E:]
    bq, bk, bv = in_proj_b[:E], in_proj_b[E:2 * E], in_proj_b[2 * E:]

    # host precompute: fold Wq/Wk into per-batch query vectors
    qh = (q_global @ Wq.T + bq).reshape(B, H, HD)
    scale = 1.0 / np.sqrt(HD)
    rq = np.einsum('bhd,hde->bhe', qh, Wk.reshape(H, HD, E)) * scale  # (B,H,E)
    sbias = (qh * bk.reshape(H, HD)[None]).sum(-1) * scale            # (B,H)
    # device computes scores without sbias; exp(s+sb) = exp(s)*exp(sb) cancels
    # in the softmax per head, and for attn_w it scales E2 and Z1 equally. So
    # sbias drops out entirely -- no need to send it.
    rqt = np.ascontiguousarray(rq.transpose(0, 2, 1))                 # (B,E,H)
    nst = np.ascontiguousarray(
        (noise * 0.05).reshape(B, NT, 128).transpose(0, 2, 1))        # (B,128,NT)

    if _NC_CACHE is None:
        _NC_CACHE = build_bass()
    nc = _NC_CACHE

    in_maps = []
    for c in range(NCORES):
        sl = slice(c * BPC, (c + 1) * BPC)
        in_maps.append({
            "v": np.ascontiguousarray(v_seq[sl]),
            "rqt": np.ascontiguousarray(rqt[sl]),
            "nst": np.ascontiguousarray(nst[sl]),
        })

    import os
    trace = bool(int(os.environ.get("KTRACE", "0")))
    res = run_bass_kernel_spmd(nc, in_maps, core_ids=list(range(NCORES)),
                               trace=trace)
    global LAST_EXEC_NS
    LAST_EXEC_NS = getattr(res, "exec_time_ns", None)
    outs = res.results

    t_dev = np.concatenate([np.asarray(outs[c]["out_t"]) for c in range(NCORES)], 0)
    z_dev = np.concatenate([np.asarray(outs[c]["out_z"]) for c in range(NCORES)], 0)
    t_dev = t_dev.reshape(B, H, E)
    z_dev = z_dev.reshape(B, H)

    # host postprocess: ctx = Wv_h @ (t_h/z_h) + bv_h ; out proj ; concat
    ctx = np.einsum('hde,bhe->bhd', Wv.reshape(H, HD, E), t_dev / z_dev[..., None]) \
        + bv.reshape(H, HD)[None]
    att = ctx.reshape(B, E) @ out_proj_w.T + out_proj_b
    return np.concatenate([att, np.asarray(q_global, np.float32)], axis=1)
